# revision 16
# baseline (speedup 1.0000x reference)
"""AttnDecoderRNN step on 8 Trainium2 NeuronCores (Bass/Tile SPMD kernel).

Sharding strategy (per the tensor-parallel hint):
  - Attention inner dim D=2H=2048 sharded 256/core: WkT & Wv row-shards; scores
    via batched dot products on DVE (batch on partitions, packed 2x64); partial
    scores AllGathered + locally combined; context partials exchanged with an
    AllToAll and locally reduced into the core's h-shard.
  - Embedding table row-sharded (vocab) across cores; local indirect-DMA gather
    + mask + AllReduce assembles the embedded vectors.
  - LSTM gate columns sharded 128/core (all 4 gates); h_next AllGathered.
  - Output projection column-sharded 6283 vocab cols/core; log_softmax via local
    exp-sums AllGathered and combined on every core.

Precision: weights are bf16 (storage + TensorE), accumulation f32; softmax,
LSTM pointwise, logits, and all outputs are f32.

kernel(**inputs) takes FULL numpy inputs, returns (log_probs, h_next, attn).
"""
import os
import sys

for _p in ("/opt/trn_rl_repo", "/root/.axon_site/_ro/trn_rl_repo"):
    if os.path.isdir(_p) and _p not in sys.path:
        sys.path.insert(0, _p)

import numpy as np
import ml_dtypes

import concourse.bass as bass
import concourse.mybir as mybir
import concourse.tile as tile
from concourse import bacc
from concourse.bass_utils import run_bass_kernel_spmd
from concourse.masks import make_identity

F32 = mybir.dt.float32
BF16 = mybir.dt.bfloat16
I32 = mybir.dt.int32
AF = mybir.ActivationFunctionType
BF = ml_dtypes.bfloat16

B, L, H, V = 64, 50, 1024, 50257
M = 8                      # cores
DS = 2 * H // M            # 256  attention-dim shard
VS = -(-V // M)            # 6283 vocab cols per core (8*6283 = 50264)
VR = VS                    # emb table rows per core
GS = 4 * 128               # 512 gate cols per core (128 per gate)
KT = H // 128              # 8 k-tiles over H
NEG_PAD = -30.0            # logit value for padded vocab columns
VCH = 1024                 # unembed chunk width

LAST_EXEC_NS = None        # test harness reads this after a traced call


def build_nc():
    nc = bacc.Bacc("TRN2", target_bir_lowering=False, debug=False, num_devices=M)

    def din(name, shape, dtype=F32):
        return nc.dram_tensor(name, shape, dtype, kind="ExternalInput")

    # ---- per-core inputs (host pre-sharded) ----
    idx_loc = din("idx_loc", [B, 1], I32)       # local emb-table row or clamp
    gmask = din("gmask", [B, 1])                # 1.0 if this core owns the row
    emb_sh = din("emb_sh", [VR, H], BF16)       # emb row-shard (zero padded)
    h_prevT = din("h_prevT", [H, B], BF16)      # h_prev transposed (bf16)
    enc_sh = din("enc_sh", [128, L, 128], BF16) # [(2 halves x 64b), L, 128d]
    wq = din("wq", [H, H], BF16)                # query_w / H (scale folded)
    bqT = din("bqT", [128, KT])                 # query_b / H
    wkt = din("wkt", [H, DS], BF16)             # key_w[shard,:]^T
    bk8 = din("bk8", [128, KT], BF16)           # key_b / M
    wv = din("wv", [DS, H], BF16)               # value_w[shard,:]
    bv_sh = din("bv_sh", [128, 1])              # value_b h-shard
    wxa = din("wxa", [H + 128, GS], BF16)       # w_x cols + bias row (augmented)
    wh = din("wh", [H, GS], BF16)               # w_h cols
    wo = din("wo", [H, VS], BF16)               # out_w vocab-col shard
    wob = din("wob", [1, VS], BF16)             # out_b shard (pad = NEG_PAD)

    # ---- outputs ----
    out_logp = nc.dram_tensor("out_logp", [B, VS], F32, kind="ExternalOutput")
    out_h = nc.dram_tensor("out_h", [128, B], F32, kind="ExternalOutput")
    out_attn = nc.dram_tensor("out_attn", [B, L], F32, kind="ExternalOutput")

    groups = [list(range(M))]
    NVC = -(-VS // VCH)  # 7 vocab chunks

    wq_ap = wq.ap().rearrange("(k p) h -> p k h", p=128)
    wkt_ap = wkt.ap().rearrange("(k p) d -> p k d", p=128)
    wv_ap = wv.ap().rearrange("(k p) h -> p k h", p=128)
    wxa_ap = wxa.ap().rearrange("(k p) j -> p k j", p=128)
    wh_ap = wh.ap().rearrange("(k p) j -> p k j", p=128)
    wo_ap = wo.ap().rearrange("(k p) v -> p k v", p=128)

    with tile.TileContext(nc) as tc:
        with (
            tc.tile_pool(name="cst", bufs=1) as cst,
            tc.tile_pool(name="wst", bufs=2) as wst,
            tc.tile_pool(name="wbufp", bufs=1) as wbufp,
            tc.tile_pool(name="ps", bufs=2, space="PSUM") as ps,
            tc.tile_pool(name="psl", bufs=2, space="PSUM") as psl,
            tc.tile_pool(name="dram", bufs=1, space="DRAM") as dram,
        ):
            # ---------- resident loads (critical-path first; enc split
            # across queues) ----------
            hpTb_sb = cst.tile([128, KT, B], BF16)
            nc.sync.dma_start(hpTb_sb[:], h_prevT.ap().rearrange("(k p) b -> p k b", p=128))
            enc_sb = cst.tile([128, L, 128], BF16)
            for i in range(4):
                l0, l1 = (L * i) // 4, (L * (i + 1)) // 4
                nc.sync.dma_start(enc_sb[:, l0:l1, :], enc_sh[:, l0:l1, :])
            bqT_sb = cst.tile([128, KT], F32)
            nc.sync.dma_start(bqT_sb[:], bqT[:])
            bk8_sb = cst.tile([128, KT], BF16)
            nc.sync.dma_start(bk8_sb[:], bk8[:])
            bv_sb = cst.tile([128, 1], F32)
            nc.sync.dma_start(bv_sb[:], bv_sh[:])
            gmask_sb = cst.tile([B, 1], F32)
            nc.sync.dma_start(gmask_sb[:], gmask[:])
            idx_sb = cst.tile([B, 1], I32)
            nc.sync.dma_start(idx_sb[:], idx_loc[:])

            ident = cst.tile([128, 128], F32)
            make_identity(nc, ident[:])
            ident_bf = cst.tile([128, 128], BF16)
            make_identity(nc, ident_bf[:])
            ones_bf = cst.tile([1, B], BF16)
            nc.vector.memset(ones_bf[:], 1.0)

            # ---------- q = h_prev @ (Wq/H)  (no bias yet) ----------
            q_ps0 = ps.tile([B, 512], F32, tag="qps")
            q_ps1 = ps.tile([B, 512], F32, tag="qps")
            for k in range(KT):
                wq_t = wst.tile([128, H], BF16, tag="wq", bufs=KT)
                nc.sync.dma_start(wq_t[:, 0:512], wq_ap[:, k, 0:512])
                nc.sync.dma_start(wq_t[:, 512:1024], wq_ap[:, k, 512:1024])
                for half, qp in ((0, q_ps0), (1, q_ps1)):
                    nc.tensor.matmul(
                        qp[:], lhsT=hpTb_sb[:, k, :], rhs=wq_t[:, half * 512:(half + 1) * 512],
                        start=(k == 0), stop=(k == KT - 1),
                    )
            q_sb = cst.tile([B, H], F32)
            nc.scalar.copy(q_sb[:, 0:512], q_ps0[:])
            nc.scalar.copy(q_sb[:, 512:1024], q_ps1[:])

            # qT (bf16) with per-partition bias bq/H
            qT_sb = cst.tile([128, KT, B], BF16)
            for k in range(KT):
                tp = ps.tile([128, B], F32, tag="tps")
                nc.tensor.transpose(tp[:], q_sb[:, k * 128:(k + 1) * 128], ident[:B, :B])
                nc.scalar.activation(qT_sb[:, k, :], tp[:], AF.Identity, bias=bqT_sb[:, k:k + 1])

            # ---------- qk packed [ (2 halves x 64b), 128 d ] ----------
            qk_ps = ps.tile([128, 128], F32, tag="qps")
            wkt_ts = []
            for k in range(KT):
                wkt_t = wst.tile([128, DS], BF16, tag="wkt", bufs=KT, name=f"wkt_{k}")
                nc.sync.dma_start(wkt_t[:], wkt_ap[:, k, :])
                wkt_ts.append(wkt_t)
                nc.tensor.matmul(qk_ps[:B, :], lhsT=qT_sb[:, k, :], rhs=wkt_t[:, 0:128],
                                 start=(k == 0), stop=(k == KT - 1))
            for k in range(KT):
                nc.tensor.matmul(qk_ps[B:, :], lhsT=qT_sb[:, k, :], rhs=wkt_ts[k][:, 128:256],
                                 start=(k == 0), stop=(k == KT - 1), tile_position=(0, 64))
            qk_sb = cst.tile([128, 128], BF16)
            nc.scalar.copy(qk_sb[:], qk_ps[:])

            # qb = q' . (key_b/M)   [B, 1]
            qb_ps = ps.tile([B, 1], F32, tag="tps")
            for k in range(KT):
                nc.tensor.matmul(qb_ps[:], lhsT=qT_sb[:, k, :], rhs=bk8_sb[:, k:k + 1],
                                 start=(k == 0), stop=(k == KT - 1))
            qb_sb = cst.tile([B, 1], F32)
            nc.scalar.copy(qb_sb[:], qb_ps[:])

            # ---------- scores partial = sum_d enc*qk (tree over d) ----------
            wbuf = wbufp.tile([128, L, 128], BF16, tag="wbuf")
            nc.vector.tensor_mul(wbuf[:], enc_sb[:], qk_sb[:, None, :].to_broadcast([128, L, 128]))
            n = 128
            while n > 1:
                hn = n // 2
                nc.vector.tensor_add(wbuf[:, :, 0:hn], wbuf[:, :, 0:hn], wbuf[:, :, hn:n])
                n = hn
            sredf = cst.tile([128, L], F32)
            nc.vector.tensor_copy(sredf[:], wbuf[:, :, 0])
            # add qb to the lower half, then fold halves via DMA-accumulate (CCE)
            nc.vector.tensor_scalar_add(sredf[:B, :], sredf[:B, :], qb_sb[:, 0:1])

            # ---------- embedding gather (fused into the scores AllGather:
            # cols [0:L) carry score partials f32, cols [L:L+H/2) carry the
            # masked bf16 embedding gather, bitcast to f32 pairs) ------------
            gat = cst.tile([B, H], BF16)
            nc.gpsimd.indirect_dma_start(
                out=gat[:], out_offset=None,
                in_=emb_sh[:],
                in_offset=bass.IndirectOffsetOnAxis(ap=idx_sb[:, :1], axis=0),
            )
            gat_m = cst.tile([B, H], BF16)
            nc.vector.tensor_scalar_mul(gat_m[:], gat[:], gmask_sb[:, 0:1])

            EW = H // 2  # 512 f32 words carrying 1024 bf16 embedding values
            cc_sc_in = dram.tile([B, L + EW], F32)
            cc_sc_out = dram.tile([M * B, L + EW], F32, addr_space="Shared")
            nc.sync.dma_start(cc_sc_in[:, L:], gat_m[:].bitcast(F32))
            nc.sync.dma_start(cc_sc_in[:, 0:L], sredf[:B, :])
            nc.gpsimd.dma_start(cc_sc_in[:, 0:L], sredf[B:, :], accum_op=mybir.AluOpType.add)
            nc.gpsimd.collective_compute(
                "AllGather", mybir.AluOpType.bypass, replica_groups=groups,
                ins=[cc_sc_in.opt()], outs=[cc_sc_out.opt()],
            )

            # ---------- combine gathered score partials + softmax ----------
            sc8 = cst.tile([128, M, L], F32)
            sc_src = cc_sc_out.rearrange("(m b) e -> b m e", b=B)
            nc.sync.dma_start(sc8[:B, :, :], sc_src[:, :, 0:L])
            nc.sync.dma_start(sc8[B:, :, :], sc_src[:, :, 0:L])

            # embedding blocks: masked per-core contributions; summing them
            # just reassembles the rows (disjoint masks -> exact in bf16)
            em8 = cst.tile([B, M, EW], F32)
            for i in range(4):
                m0, m1 = 2 * i, 2 * i + 2
                nc.sync.dma_start(em8[:, m0:m1, :], sc_src[:, m0:m1, L:])
            em8b = em8[:].bitcast(BF16)           # [B, M, H]
            n = M
            while n > 1:
                hn = n // 2
                nc.vector.tensor_add(em8b[:, 0:hn, :], em8b[:, 0:hn, :], em8b[:, hn:n, :])
                n = hn
            embedded = em8b[:, 0, :]              # [B, H] bf16 view
            n = M
            while n > 1:
                hn = n // 2
                nc.vector.tensor_add(sc8[:, 0:hn, :], sc8[:, 0:hn, :], sc8[:, hn:n, :])
                n = hn
            attn_raw = sc8[:, 0, :]
            nmax = cst.tile([128, 1], F32)
            nc.vector.reduce_max(nmax[:], attn_raw, axis=mybir.AxisListType.X, negate=True)
            attn_e = cst.tile([128, L], F32)
            sexp = cst.tile([128, 1], F32)
            nc.scalar.activation(attn_e[:], attn_raw, AF.Exp,
                                 bias=nmax[:, 0:1], accum_out=sexp[:, 0:1])
            rcp = cst.tile([128, 1], F32)
            nc.vector.reciprocal(rcp[:], sexp[:])
            attn_bf = cst.tile([128, L], BF16)
            nc.vector.tensor_scalar_mul(attn_bf[:], attn_e[:], rcp[:, 0:1])
            attn_f = cst.tile([B, L], F32)
            nc.vector.tensor_scalar_mul(attn_f[:], attn_e[:B, :], rcp[:B, 0:1])
            nc.sync.dma_start(out_attn[:], attn_f[:])

            # ---------- t = attn @ enc  (packed, tree reduce over L) ----------
            wbuf2 = wbufp.tile([128, L, 128], BF16, tag="wbuf")
            nc.vector.tensor_mul(wbuf2[:], enc_sb[:],
                                 attn_bf[:, :, None].to_broadcast([128, L, 128]))
            n = L
            while n > 1:
                hn = n // 2
                r = n - hn
                nc.vector.tensor_add(wbuf2[:, 0:hn, :], wbuf2[:, 0:hn, :], wbuf2[:, r:n, :])
                n = r

            # tT [d(2x128), b]  (bf16)
            tT_sb = cst.tile([128, 2, B], BF16)
            for g in range(2):
                tpb = ps.tile([128, B], BF16, tag="tps", name=f"tpb_{g}")
                nc.tensor.transpose(tpb[:], wbuf2[g * B:(g + 1) * B, 0, :],
                                    ident_bf[g * B:(g + 1) * B, g * B:(g + 1) * B])
                nc.scalar.copy(tT_sb[:, g, :], tpb[:])

            # ---------- cT partial -> AllToAll -> local reduce ----------
            ctb_sb = cst.tile([128, KT, B], F32)
            for mo in range(KT):
                cp = ps.tile([128, B], F32, tag="tps", name=f"cp_{mo}")
                for kd in range(2):
                    wv_t = wst.tile([128, 128], BF16, tag="wv", bufs=4, name=f"wv_{mo}_{kd}")
                    nc.sync.dma_start(wv_t[:], wv_ap[:, kd, mo * 128:(mo + 1) * 128])
                    nc.tensor.matmul(cp[:], lhsT=wv_t[:], rhs=tT_sb[:, kd, :],
                                     start=(kd == 0), stop=(kd == 1))
                nc.scalar.copy(ctb_sb[:, mo, :], cp[:])

            cc_ct_in = dram.tile([H, B], F32)
            cc_ct_out = dram.tile([H, B], F32)
            ct_dst = cc_ct_in.rearrange("(k p) b -> p k b", p=128)
            for i in range(4):
                k0, k1 = 2 * i, 2 * i + 2
                nc.sync.dma_start(ct_dst[:, k0:k1, :], ctb_sb[:, k0:k1, :])
            nc.gpsimd.collective_compute(
                "AllToAll", mybir.AluOpType.bypass, replica_groups=groups,
                ins=[cc_ct_in.opt()], outs=[cc_ct_out.opt()],
            )
            ct8_sb = cst.tile([128, M, B], F32)
            for i in range(4):
                m0, m1 = 2 * i, 2 * i + 2
                nc.sync.dma_start(
                    ct8_sb[:, m0:m1, :],
                    cc_ct_out.rearrange("(m p) b -> p m b", p=128)[:, m0:m1, :])
            n = M
            while n > 1:
                hn = n // 2
                nc.vector.tensor_add(ct8_sb[:, 0:hn, :], ct8_sb[:, 0:hn, :], ct8_sb[:, hn:n, :])
                n = hn
            ct_b = cst.tile([128, B], F32)
            nc.scalar.activation(ct_b[:], ct8_sb[:, 0, :], AF.Identity, bias=bv_sb[:, 0:1])

            # c_prev shard in [b, j] layout
            cb_ps = ps.tile([B, 128], F32, tag="tps")
            nc.tensor.transpose(cb_ps[:], ct_b[:], ident[:, :])
            cb_sb = cst.tile([B, 128], F32)
            nc.scalar.copy(cb_sb[:], cb_ps[:])

            # ---------- embT build (bf16) ----------
            embT_sb = cst.tile([128, KT + 1, B], BF16)
            for k in range(KT):
                tpe = ps.tile([128, B], BF16, tag="tps", name=f"tpe_{k}")
                nc.tensor.transpose(tpe[:], embedded[:, k * 128:(k + 1) * 128], ident_bf[:B, :B])
                nc.scalar.copy(embT_sb[:, k, :], tpe[:])
            nc.vector.memset(embT_sb[:, KT, :], 0.0)
            nc.vector.memset(embT_sb[0:1, KT, :], 1.0)

            # ---------- gates [b, 4x128] ----------
            g_ps = ps.tile([B, GS], F32, tag="qps")
            for k in range(KT + 1):
                wxa_t = wst.tile([128, GS], BF16, tag="wg", bufs=3)
                nc.sync.dma_start(wxa_t[:], wxa_ap[:, k, :])
                nc.tensor.matmul(g_ps[:], lhsT=embT_sb[:, k, :], rhs=wxa_t[:],
                                 start=(k == 0), stop=False)
            for k in range(KT):
                wh_t = wst.tile([128, GS], BF16, tag="wg", bufs=3)
                nc.sync.dma_start(wh_t[:], wh_ap[:, k, :])
                nc.tensor.matmul(g_ps[:], lhsT=hpTb_sb[:, k, :], rhs=wh_t[:],
                                 start=False, stop=(k == KT - 1))
            g_sb = cst.tile([B, GS], F32)
            nc.scalar.copy(g_sb[:], g_ps[:])

            f_s = cst.tile([B, 128], F32)
            i_s = cst.tile([B, 128], F32)
            cbar = cst.tile([B, 128], F32)
            o_s = cst.tile([B, 128], F32)
            nc.scalar.activation(f_s[:], g_sb[:, 0:128], AF.Sigmoid)
            nc.scalar.activation(i_s[:], g_sb[:, 128:256], AF.Sigmoid)
            nc.scalar.activation(cbar[:], g_sb[:, 256:384], AF.Tanh)
            nc.scalar.activation(o_s[:], g_sb[:, 384:512], AF.Sigmoid)

            t1 = cst.tile([B, 128], F32)
            nc.vector.tensor_mul(t1[:], f_s[:], cb_sb[:])
            t2 = cst.tile([B, 128], F32)
            nc.vector.tensor_mul(t2[:], i_s[:], cbar[:])
            cn = cst.tile([B, 128], F32)
            nc.vector.tensor_add(cn[:], t1[:], t2[:])
            tc_ = cst.tile([B, 128], F32)
            nc.scalar.activation(tc_[:], cn[:], AF.Tanh)
            hs_ = cst.tile([B, 128], F32)
            nc.vector.tensor_mul(hs_[:], o_s[:], tc_[:])

            hT_ps = ps.tile([128, B], F32, tag="tps")
            nc.tensor.transpose(hT_ps[:], hs_[:], ident[:B, :B])
            hT_sh = cst.tile([128, B], F32)
            nc.scalar.copy(hT_sh[:], hT_ps[:])
            nc.sync.dma_start(out_h[:], hT_sh[:])

            cc_h_in = dram.tile([128, B], F32)
            cc_h_out = dram.tile([H, B], F32, addr_space="Shared")
            nc.sync.dma_start(cc_h_in[:], hT_sh[:])
            nc.gpsimd.collective_compute(
                "AllGather", mybir.AluOpType.bypass, replica_groups=groups,
                ins=[cc_h_in.opt()], outs=[cc_h_out.opt()],
            )
            hT_sb = cst.tile([128, KT, B], F32)
            h_src = cc_h_out.rearrange("(k p) b -> p k b", p=128)
            for i in range(4):
                k0, k1 = 2 * i, 2 * i + 2
                nc.sync.dma_start(hT_sb[:, k0:k1, :], h_src[:, k0:k1, :])
            hTb_sb = cst.tile([128, KT, B], BF16)
            nc.vector.tensor_copy(hTb_sb[:], hT_sb[:])

            # ---------- unembed (bf16): logits chunks + exp sums ----------
            lgs_sb = cst.tile([B, VS], F32)
            sums = cst.tile([B, NVC], F32)
            for j in range(NVC):
                nj = min(VCH, VS - j * VCH)
                wo_t = wst.tile([128, KT, VCH], BF16, tag="wo", bufs=4)
                for i in range(4):
                    k0, k1 = 2 * i, 2 * i + 2
                    nc.sync.dma_start(wo_t[:, k0:k1, :nj],
                                      wo_ap[:, k0:k1, j * VCH:j * VCH + nj])
                wob_t = wst.tile([1, VCH], BF16, tag="wob", bufs=2)
                nc.sync.dma_start(wob_t[:, :nj], wob[:, j * VCH:j * VCH + nj])
                lg_ps = psl.tile([B, VCH], F32, tag="lgps")
                for h0 in range(0, nj, 512):
                    h1 = min(nj, h0 + 512)
                    for k in range(KT):
                        nc.tensor.matmul(lg_ps[:, h0:h1], lhsT=hTb_sb[:, k, :],
                                         rhs=wo_t[:, k, h0:h1], start=(k == 0), stop=False)
                    nc.tensor.matmul(lg_ps[:, h0:h1], lhsT=ones_bf[:], rhs=wob_t[:, h0:h1],
                                     start=False, stop=True)
                nc.scalar.copy(lgs_sb[:, j * VCH:j * VCH + nj], lg_ps[:, :nj])
                scr = wst.tile([B, VCH], F32, tag="scr", bufs=2)
                nc.scalar.activation(scr[:, :nj], lg_ps[:, :nj], AF.Exp,
                                     accum_out=sums[:, j:j + 1])

            # ---------- global logsumexp ----------
            ssum = cst.tile([B, 1], F32)
            nc.vector.reduce_sum(ssum[:], sums[:, 0:NVC], axis=mybir.AxisListType.X)
            cc_st_in = dram.tile([B, 1], F32)
            cc_st_out = dram.tile([M * B, 1], F32, addr_space="Shared")
            nc.sync.dma_start(cc_st_in[:], ssum[:])
            nc.gpsimd.collective_compute(
                "AllGather", mybir.AluOpType.bypass, replica_groups=groups,
                ins=[cc_st_in.opt()], outs=[cc_st_out.opt()],
            )
            st_sb = cst.tile([B, M], F32)
            nc.sync.dma_start(st_sb[:], cc_st_out.rearrange("(m b) o -> b (m o)", b=B))
            gsum = cst.tile([B, 1], F32)
            nc.vector.reduce_sum(gsum[:], st_sb[:], axis=mybir.AxisListType.X)
            lse = cst.tile([B, 1], F32)
            nc.scalar.activation(lse[:], gsum[:], AF.Ln)
            nlse = cst.tile([B, 1], F32)
            nc.vector.tensor_scalar_mul(nlse[:], lse[:], -1.0)

            # ---------- final: log_probs = logits - lse (DVE/ACT split) ------
            nsl = 8
            step = -(-VS // nsl)
            for s in range(nsl):
                lo = s * step
                hi = min(VS, lo + step)
                if s % 2 == 0:
                    nc.vector.tensor_scalar_add(lgs_sb[:, lo:hi], lgs_sb[:, lo:hi],
                                                nlse[:, 0:1])
                else:
                    nc.scalar.activation(lgs_sb[:, lo:hi], lgs_sb[:, lo:hi],
                                         AF.Identity, bias=nlse[:, 0:1])
                nc.sync.dma_start(out_logp[:, lo:hi], lgs_sb[:, lo:hi])

    nc.compile()
    return nc


def make_in_maps(input, h_prev, encoder_outputs, emb, w_x, b_x, w_h, b_h,
                 key_w, key_b, value_w, value_b, query_w, query_b, out_w, out_b):
    f = lambda x: np.ascontiguousarray(np.asarray(x, dtype=np.float32))
    bf = lambda x: np.ascontiguousarray(np.asarray(x, dtype=np.float32).astype(BF))
    idx = np.asarray(input).reshape(B).astype(np.int64)
    h_prev = f(h_prev).reshape(B, H)
    enc = f(encoder_outputs)
    w_x, b_x, w_h, b_h = f(w_x), f(b_x), f(w_h), f(b_h)
    key_w, key_b = f(key_w), f(key_b)
    value_w, value_b = f(value_w), f(value_b)
    query_w, query_b = f(query_w), f(query_b)
    out_w, out_b = f(out_w), f(out_b)

    h_prevT = bf(h_prev.T)                                         # [H, B]
    wq_s = bf(query_w / H)
    bqT = np.ascontiguousarray((query_b / H).reshape(KT, 128).T)   # [128, KT]
    bk8 = bf((key_b / M).reshape(KT, 128).T)
    bxh = b_x + b_h
    emb_bf = np.asarray(emb, dtype=np.float32).astype(BF)

    in_maps = []
    for m in range(M):
        # embedding row shard
        r0 = m * VR
        nrow = max(0, min(VR, V - r0))
        emb_sh = np.zeros((VR, H), BF)
        emb_sh[:nrow] = emb_bf[r0:r0 + nrow]
        own = (idx >= r0) & (idx < r0 + nrow)
        idx_loc = np.clip(idx - r0, 0, VR - 1).astype(np.int32).reshape(B, 1)
        gmask = own.astype(np.float32).reshape(B, 1)

        # attention shard
        s = m * DS
        enc_sh = bf(np.concatenate([enc[:, :, s:s + 128], enc[:, :, s + 128:s + DS]], axis=0))
        wkt = bf(key_w[s:s + DS, :].T)                             # [H, DS]
        wv = bf(value_w[s:s + DS, :])                              # [DS, H]
        bv_sh = np.ascontiguousarray(value_b[m * 128:(m + 1) * 128].reshape(128, 1))

        # gate column shard
        cols = np.concatenate([np.arange(g * H + m * 128, g * H + (m + 1) * 128)
                               for g in range(4)])
        wxa = np.zeros((H + 128, GS), BF)
        wxa[:H] = w_x[:, cols].astype(BF)
        wxa[H] = bxh[cols].astype(BF)
        wh_sh = bf(w_h[:, cols])

        # vocab column shard
        c0 = m * VS
        ncol = max(0, min(VS, V - c0))
        wo = np.zeros((H, VS), BF)
        wo[:, :ncol] = out_w[:, c0:c0 + ncol].astype(BF)
        wob = np.full((1, VS), NEG_PAD, BF)
        wob[0, :ncol] = out_b[c0:c0 + ncol].astype(BF)

        in_maps.append({
            "idx_loc": idx_loc, "gmask": gmask, "emb_sh": emb_sh,
            "h_prevT": h_prevT, "enc_sh": enc_sh,
            "wq": wq_s, "bqT": bqT, "wkt": wkt, "bk8": bk8,
            "wv": wv, "bv_sh": bv_sh, "wxa": wxa, "wh": wh_sh,
            "wo": wo, "wob": wob,
        })
    return in_maps


_NC_CACHE = None


def _get_nc():
    global _NC_CACHE
    if _NC_CACHE is None:
        _NC_CACHE = build_nc()
    return _NC_CACHE


def kernel(**inputs):
    global LAST_EXEC_NS
    nc = _get_nc()
    in_maps = make_in_maps(**inputs)
    trace = bool(int(os.environ.get("KERNEL_TRACE", "0")))
    res = run_bass_kernel_spmd(nc, in_maps, core_ids=list(range(M)), trace=trace)
    LAST_EXEC_NS = res.exec_time_ns
    outs = res.results

    logp = np.concatenate([outs[m]["out_logp"] for m in range(M)], axis=1)[:, :V]
    hT = np.concatenate([outs[m]["out_h"] for m in range(M)], axis=0)   # [H, B]
    attn = outs[0]["out_attn"]
    return (
        logp.reshape(B, 1, V).astype(np.float32),
        np.ascontiguousarray(hT.T).reshape(B, 1, H).astype(np.float32),
        attn.reshape(B, 1, L).astype(np.float32),
    )


if __name__ == "__main__":
    from ref_np import setup_inputs_np
    ins = setup_inputs_np(0)
    outs = kernel(**ins)
    print([o.shape for o in outs], "exec_ns:", LAST_EXEC_NS)


# revision 25
# speedup vs baseline: 1.0114x; 1.0114x over previous
"""AttnDecoderRNN step on 8 Trainium2 NeuronCores (Bass/Tile SPMD kernel).

Sharding strategy (per the tensor-parallel hint):
  - Attention inner dim D=2H=2048 sharded 256/core: WkT & Wv row-shards; scores
    via batched dot products on DVE (batch on partitions, packed 2x64); partial
    scores AllGathered + locally combined; context partials exchanged with an
    AllToAll and locally reduced into the core's h-shard.
  - Embedding table row-sharded (vocab) across cores; local indirect-DMA gather
    + mask + AllReduce assembles the embedded vectors.
  - LSTM gate columns sharded 128/core (all 4 gates); h_next AllGathered.
  - Output projection column-sharded 6283 vocab cols/core; log_softmax via local
    exp-sums AllGathered and combined on every core.

Precision: weights are bf16 (storage + TensorE), accumulation f32; softmax,
LSTM pointwise, logits, and all outputs are f32.

kernel(**inputs) takes FULL numpy inputs, returns (log_probs, h_next, attn).
"""
import os
import sys

for _p in ("/opt/trn_rl_repo", "/root/.axon_site/_ro/trn_rl_repo"):
    if os.path.isdir(_p) and _p not in sys.path:
        sys.path.insert(0, _p)

import numpy as np
import ml_dtypes

import concourse.bass as bass
import concourse.mybir as mybir
import concourse.tile as tile
from concourse import bacc
from concourse.bass_utils import run_bass_kernel_spmd
from concourse.masks import make_identity

F32 = mybir.dt.float32
BF16 = mybir.dt.bfloat16
I32 = mybir.dt.int32
AF = mybir.ActivationFunctionType
BF = ml_dtypes.bfloat16

B, L, H, V = 64, 50, 1024, 50257
M = 8                      # cores
DS = 2 * H // M            # 256  attention-dim shard
VS = -(-V // M)            # 6283 vocab cols per core (8*6283 = 50264)
VR = VS                    # emb table rows per core
GS = 4 * 128               # 512 gate cols per core (128 per gate)
KT = H // 128              # 8 k-tiles over H
NEG_PAD = -30.0            # logit value for padded vocab columns
VCH = 1024                 # unembed chunk width

LAST_EXEC_NS = None        # test harness reads this after a traced call


def build_nc():
    nc = bacc.Bacc("TRN2", target_bir_lowering=False, debug=False, num_devices=M)

    def din(name, shape, dtype=F32):
        return nc.dram_tensor(name, shape, dtype, kind="ExternalInput")

    # ---- per-core inputs (host pre-sharded) ----
    idx_loc = din("idx_loc", [B, 1], I32)       # local emb-table row or clamp
    gmask = din("gmask", [B, 1])                # 1.0 if this core owns the row
    emb_sh = din("emb_sh", [VR, H], BF16)       # emb row-shard (zero padded)
    h_prevT = din("h_prevT", [H, B], BF16)      # h_prev transposed (bf16)
    enc_sh = din("enc_sh", [128, L, 128], BF16) # [(2 halves x 64b), L, 128d]
    wq = din("wq", [H, H], BF16)                # query_w / H (scale folded)
    bqT = din("bqT", [128, KT])                 # query_b / H
    wkt = din("wkt", [H, DS], BF16)             # key_w[shard,:]^T
    bk8 = din("bk8", [128, KT], BF16)           # key_b / M
    wv = din("wv", [DS, H], BF16)               # value_w[shard,:]
    bv_sh = din("bv_sh", [128, 1])              # value_b h-shard
    wxa = din("wxa", [H + 128, GS], BF16)       # w_x cols + bias row (augmented)
    wh = din("wh", [H, GS], BF16)               # w_h cols
    wo = din("wo", [H, VS], BF16)               # out_w vocab-col shard
    wob = din("wob", [1, VS], BF16)             # out_b shard (pad = NEG_PAD)
    fold_in = din("fold_in", [128, B], BF16)    # [I_64; I_64] partition-fold matrix

    # ---- outputs ----
    out_logp = nc.dram_tensor("out_logp", [B, VS], F32, kind="ExternalOutput")
    out_h = nc.dram_tensor("out_h", [128, B], F32, kind="ExternalOutput")
    out_attn = nc.dram_tensor("out_attn", [B, L], F32, kind="ExternalOutput")

    groups = [list(range(M))]
    NVC = -(-VS // VCH)  # 7 vocab chunks

    wq_ap = wq.ap().rearrange("(k p) h -> p k h", p=128)
    wkt_ap = wkt.ap().rearrange("(k p) d -> p k d", p=128)
    wv_ap = wv.ap().rearrange("(k p) h -> p k h", p=128)
    wxa_ap = wxa.ap().rearrange("(k p) j -> p k j", p=128)
    wh_ap = wh.ap().rearrange("(k p) j -> p k j", p=128)
    wo_ap = wo.ap().rearrange("(k p) v -> p k v", p=128)

    with tile.TileContext(nc) as tc:
        with (
            tc.tile_pool(name="cst", bufs=1) as cst,
            tc.tile_pool(name="wst", bufs=2) as wst,
            tc.tile_pool(name="wbufp", bufs=1) as wbufp,
            tc.tile_pool(name="ps", bufs=2, space="PSUM") as ps,
            tc.tile_pool(name="psl", bufs=2, space="PSUM") as psl,
            tc.tile_pool(name="dram", bufs=1, space="DRAM") as dram,
        ):
            # ---------- resident loads (critical-path first; enc split
            # across queues) ----------
            hpTb_sb = cst.tile([128, KT, B], BF16)
            nc.sync.dma_start(hpTb_sb[:], h_prevT.ap().rearrange("(k p) b -> p k b", p=128))
            enc_sb = cst.tile([128, L, 128], BF16)
            for i in range(8):
                l0, l1 = (L * i) // 8, (L * (i + 1)) // 8
                nc.sync.dma_start(enc_sb[:, l0:l1, :], enc_sh[:, l0:l1, :])
            fold_sb = cst.tile([128, B], BF16)
            nc.sync.dma_start(fold_sb[:], fold_in[:])
            bqT_sb = cst.tile([128, KT], F32)
            nc.sync.dma_start(bqT_sb[:], bqT[:])
            bk8_sb = cst.tile([128, KT], BF16)
            nc.sync.dma_start(bk8_sb[:], bk8[:])
            bv_sb = cst.tile([128, 1], F32)
            nc.sync.dma_start(bv_sb[:], bv_sh[:])
            gmask_sb = cst.tile([B, 1], F32)
            nc.sync.dma_start(gmask_sb[:], gmask[:])
            idx_sb = cst.tile([B, 1], I32)
            nc.sync.dma_start(idx_sb[:], idx_loc[:])

            ident = cst.tile([128, 128], F32)
            make_identity(nc, ident[:])
            ident_bf = cst.tile([128, 128], BF16)
            make_identity(nc, ident_bf[:])
            ones_bf = cst.tile([1, B], BF16)
            nc.vector.memset(ones_bf[:], 1.0)

            # ---------- q = h_prev @ (Wq/H)  (no bias yet) ----------
            q_ps0 = ps.tile([B, 512], F32, tag="qps")
            q_ps1 = ps.tile([B, 512], F32, tag="qps")
            for k in range(KT):
                wq_t = wst.tile([128, H], BF16, tag="wq", bufs=KT)
                nc.sync.dma_start(wq_t[:, 0:512], wq_ap[:, k, 0:512])
                nc.sync.dma_start(wq_t[:, 512:1024], wq_ap[:, k, 512:1024])
                for half, qp in ((0, q_ps0), (1, q_ps1)):
                    nc.tensor.matmul(
                        qp[:], lhsT=hpTb_sb[:, k, :], rhs=wq_t[:, half * 512:(half + 1) * 512],
                        start=(k == 0), stop=(k == KT - 1),
                    )
            q_sb = cst.tile([B, H], F32)
            nc.scalar.copy(q_sb[:, 0:512], q_ps0[:])
            nc.scalar.copy(q_sb[:, 512:1024], q_ps1[:])

            # qT (bf16) with per-partition bias bq/H
            qT_sb = cst.tile([128, KT, B], BF16)
            for k in range(KT):
                tp = ps.tile([128, B], F32, tag="tps")
                nc.tensor.transpose(tp[:], q_sb[:, k * 128:(k + 1) * 128], ident[:B, :B])
                nc.scalar.activation(qT_sb[:, k, :], tp[:], AF.Identity, bias=bqT_sb[:, k:k + 1])

            # ---------- qk packed [ (2 halves x 64b), 128 d ] ----------
            qk_ps = ps.tile([128, 128], F32, tag="qps")
            wkt_ts = []
            for k in range(KT):
                wkt_t = wst.tile([128, DS], BF16, tag="wkt", bufs=KT, name=f"wkt_{k}")
                nc.sync.dma_start(wkt_t[:], wkt_ap[:, k, :])
                wkt_ts.append(wkt_t)
                nc.tensor.matmul(qk_ps[:B, :], lhsT=qT_sb[:, k, :], rhs=wkt_t[:, 0:128],
                                 start=(k == 0), stop=(k == KT - 1))
            for k in range(KT):
                nc.tensor.matmul(qk_ps[B:, :], lhsT=qT_sb[:, k, :], rhs=wkt_ts[k][:, 128:256],
                                 start=(k == 0), stop=(k == KT - 1), tile_position=(0, 64))
            qk_sb = cst.tile([128, 128], BF16)
            qk_copy_inst = nc.scalar.copy(qk_sb[:], qk_ps[:])

            # qb = q' . (key_b/M)   [B, 1]
            qb_ps = ps.tile([B, 1], F32, tag="tps")
            for k in range(KT):
                nc.tensor.matmul(qb_ps[:], lhsT=qT_sb[:, k, :], rhs=bk8_sb[:, k:k + 1],
                                 start=(k == 0), stop=(k == KT - 1))
            qb_sb = cst.tile([B, 1], F32)
            nc.scalar.copy(qb_sb[:], qb_ps[:])

            # ---------- scores partial = sum_d enc*qk (tree over d) ----------
            wbuf = wbufp.tile([128, L, 128], BF16, tag="wbuf")
            nc.vector.tensor_mul(wbuf[:], enc_sb[:], qk_sb[:, None, :].to_broadcast([128, L, 128]))
            n = 128
            while n > 1:
                hn = n // 2
                nc.vector.tensor_add(wbuf[:, :, 0:hn], wbuf[:, :, 0:hn], wbuf[:, :, hn:n])
                n = hn
            sred_c = cst.tile([128, L], BF16)
            nc.vector.tensor_copy(sred_c[:], wbuf[:, :, 0])
            # fold the two packed d-halves across partitions with a constant
            # [I;I] matmul, and add qb via the ACT copy bias
            fold_ps = ps.tile([B, L], F32, tag="tps")
            nc.tensor.matmul(fold_ps[:], lhsT=fold_sb[:], rhs=sred_c[:], start=True, stop=True)
            sc_part = cst.tile([B, L], F32)
            nc.scalar.activation(sc_part[:], fold_ps[:], AF.Identity, bias=qb_sb[:, 0:1])

            # ---------- embedding gather (fused into the scores AllGather:
            # cols [0:L) carry score partials f32, cols [L:L+H/2) carry the
            # masked bf16 embedding gather, bitcast to f32 pairs) ------------
            gat = cst.tile([B, H], BF16)
            nc.gpsimd.indirect_dma_start(
                out=gat[:], out_offset=None,
                in_=emb_sh[:],
                in_offset=bass.IndirectOffsetOnAxis(ap=idx_sb[:, :1], axis=0),
            )
            gat_m = cst.tile([B, H], BF16)
            nc.vector.tensor_scalar_mul(gat_m[:], gat[:], gmask_sb[:, 0:1])

            EW = H // 2  # 512 f32 words carrying 1024 bf16 embedding values
            cc_sc_in = dram.tile([B, L + EW], F32)
            cc_sc_out = dram.tile([M * B, L + EW], F32, addr_space="Shared")
            nc.sync.dma_start(cc_sc_in[:, L:], gat_m[:].bitcast(F32))
            nc.sync.dma_start(cc_sc_in[:, 0:L], sc_part[:])
            nc.gpsimd.collective_compute(
                "AllGather", mybir.AluOpType.bypass, replica_groups=groups,
                ins=[cc_sc_in.opt()], outs=[cc_sc_out.opt()],
            )

            # ---------- combine gathered score partials + softmax ----------
            sc8 = cst.tile([128, M, L], F32)
            sc_src = cc_sc_out.rearrange("(m b) e -> b m e", b=B)
            nc.sync.dma_start(sc8[:B, :, :], sc_src[:, :, 0:L])
            nc.sync.dma_start(sc8[B:, :, :], sc_src[:, :, 0:L])

            n = M
            while n > 1:
                hn = n // 2
                nc.vector.tensor_add(sc8[:, 0:hn, :], sc8[:, 0:hn, :], sc8[:, hn:n, :])
                n = hn
            attn_raw = sc8[:, 0, :]
            nmax = cst.tile([128, 1], F32)
            nc.vector.reduce_max(nmax[:], attn_raw, axis=mybir.AxisListType.X, negate=True)
            attn_e = cst.tile([128, L], F32)
            sexp = cst.tile([128, 1], F32)
            nc.scalar.activation(attn_e[:], attn_raw, AF.Exp,
                                 bias=nmax[:, 0:1], accum_out=sexp[:, 0:1])
            rcp = cst.tile([128, 1], F32)
            nc.vector.reciprocal(rcp[:], sexp[:])
            attn_bf = cst.tile([128, L], BF16)
            nc.vector.tensor_scalar_mul(attn_bf[:], attn_e[:], rcp[:, 0:1])
            attn_f = cst.tile([B, L], F32)
            nc.vector.tensor_scalar_mul(attn_f[:], attn_e[:B, :], rcp[:B, 0:1])
            nc.sync.dma_start(out_attn[:], attn_f[:])

            # ---------- t = attn @ enc  (packed, tree reduce over L) ----------
            wbuf2 = wbufp.tile([128, L, 128], BF16, tag="wbuf")
            nc.vector.tensor_mul(wbuf2[:], enc_sb[:],
                                 attn_bf[:, :, None].to_broadcast([128, L, 128]))
            n = L
            while n > 1:
                hn = n // 2
                r = n - hn
                nc.vector.tensor_add(wbuf2[:, 0:hn, :], wbuf2[:, 0:hn, :], wbuf2[:, r:n, :])
                n = r

            # tT [d(2x128), b]  (bf16)
            tT_sb = cst.tile([128, 2, B], BF16)
            for g in range(2):
                tpb = ps.tile([128, B], BF16, tag="tps", name=f"tpb_{g}")
                nc.tensor.transpose(tpb[:], wbuf2[g * B:(g + 1) * B, 0, :],
                                    ident_bf[g * B:(g + 1) * B, g * B:(g + 1) * B])
                nc.scalar.copy(tT_sb[:, g, :], tpb[:])

            # embedding blocks from the fused AllGather: masked per-core
            # contributions; summing just reassembles rows (exact in bf16)
            em8 = cst.tile([B, M, EW], F32)
            for i in range(4):
                m0, m1 = 2 * i, 2 * i + 2
                nc.sync.dma_start(em8[:, m0:m1, :], sc_src[:, m0:m1, L:])
            em8b = em8[:].bitcast(BF16)           # [B, M, H]
            n = M
            while n > 1:
                hn = n // 2
                nc.vector.tensor_add(em8b[:, 0:hn, :], em8b[:, 0:hn, :], em8b[:, hn:n, :])
                n = hn
            embedded = em8b[:, 0, :]              # [B, H] bf16 view

            # ---------- cT partial -> AllToAll -> local reduce ----------
            ctb_sb = cst.tile([128, KT, B], F32)
            for mo in range(KT):
                cp = ps.tile([128, B], F32, tag="tps", name=f"cp_{mo}")
                for kd in range(2):
                    wv_t = wst.tile([128, 128], BF16, tag="wv", bufs=4, name=f"wv_{mo}_{kd}")
                    nc.sync.dma_start(wv_t[:], wv_ap[:, kd, mo * 128:(mo + 1) * 128])
                    nc.tensor.matmul(cp[:], lhsT=wv_t[:], rhs=tT_sb[:, kd, :],
                                     start=(kd == 0), stop=(kd == 1))
                nc.scalar.copy(ctb_sb[:, mo, :], cp[:])

            cc_ct_in = dram.tile([H, B], F32)
            cc_ct_out = dram.tile([H, B], F32)
            ct_dst = cc_ct_in.rearrange("(k p) b -> p k b", p=128)
            for i in range(4):
                k0, k1 = 2 * i, 2 * i + 2
                nc.sync.dma_start(ct_dst[:, k0:k1, :], ctb_sb[:, k0:k1, :])
            nc.gpsimd.collective_compute(
                "AllToAll", mybir.AluOpType.bypass, replica_groups=groups,
                ins=[cc_ct_in.opt()], outs=[cc_ct_out.opt()],
            )
            ct8_sb = cst.tile([128, M, B], F32)
            for i in range(4):
                m0, m1 = 2 * i, 2 * i + 2
                nc.sync.dma_start(
                    ct8_sb[:, m0:m1, :],
                    cc_ct_out.rearrange("(m p) b -> p m b", p=128)[:, m0:m1, :])
            n = M
            while n > 1:
                hn = n // 2
                nc.vector.tensor_add(ct8_sb[:, 0:hn, :], ct8_sb[:, 0:hn, :], ct8_sb[:, hn:n, :])
                n = hn
            ct_b = cst.tile([128, B], F32)
            nc.scalar.activation(ct_b[:], ct8_sb[:, 0, :], AF.Identity, bias=bv_sb[:, 0:1])

            # c_prev shard in [b, j] layout
            cb_ps = ps.tile([B, 128], F32, tag="tps")
            nc.tensor.transpose(cb_ps[:], ct_b[:], ident[:, :])
            cb_sb = cst.tile([B, 128], F32)
            nc.scalar.copy(cb_sb[:], cb_ps[:])

            # ---------- embT build (bf16) ----------
            embT_sb = cst.tile([128, KT + 1, B], BF16)
            for k in range(KT):
                tpe = ps.tile([128, B], BF16, tag="tps", name=f"tpe_{k}")
                nc.tensor.transpose(tpe[:], embedded[:, k * 128:(k + 1) * 128], ident_bf[:B, :B])
                nc.scalar.copy(embT_sb[:, k, :], tpe[:])
            nc.vector.memset(embT_sb[:, KT, :], 0.0)
            nc.vector.memset(embT_sb[0:1, KT, :], 1.0)

            # ---------- gates [b, 4x128] ----------
            g_ps = ps.tile([B, GS], F32, tag="qps")
            for k in range(KT + 1):
                wxa_t = wst.tile([128, GS], BF16, tag="wg", bufs=3)
                nc.sync.dma_start(wxa_t[:], wxa_ap[:, k, :])
                nc.tensor.matmul(g_ps[:], lhsT=embT_sb[:, k, :], rhs=wxa_t[:],
                                 start=(k == 0), stop=False)
            for k in range(KT):
                wh_t = wst.tile([128, GS], BF16, tag="wg", bufs=3)
                nc.sync.dma_start(wh_t[:], wh_ap[:, k, :])
                nc.tensor.matmul(g_ps[:], lhsT=hpTb_sb[:, k, :], rhs=wh_t[:],
                                 start=False, stop=(k == KT - 1))
            g_sb = cst.tile([B, GS], F32)
            nc.scalar.copy(g_sb[:], g_ps[:])

            f_s = cst.tile([B, 128], F32)
            i_s = cst.tile([B, 128], F32)
            cbar = cst.tile([B, 128], F32)
            o_s = cst.tile([B, 128], F32)
            nc.scalar.activation(f_s[:], g_sb[:, 0:128], AF.Sigmoid)
            nc.scalar.activation(i_s[:], g_sb[:, 128:256], AF.Sigmoid)
            nc.scalar.activation(cbar[:], g_sb[:, 256:384], AF.Tanh)
            nc.scalar.activation(o_s[:], g_sb[:, 384:512], AF.Sigmoid)

            t1 = cst.tile([B, 128], F32)
            nc.vector.tensor_mul(t1[:], f_s[:], cb_sb[:])
            t2 = cst.tile([B, 128], F32)
            nc.vector.tensor_mul(t2[:], i_s[:], cbar[:])
            cn = cst.tile([B, 128], F32)
            nc.vector.tensor_add(cn[:], t1[:], t2[:])
            tc_ = cst.tile([B, 128], F32)
            nc.scalar.activation(tc_[:], cn[:], AF.Tanh)
            hs_ = cst.tile([B, 128], F32)
            nc.vector.tensor_mul(hs_[:], o_s[:], tc_[:])

            hT_ps = ps.tile([128, B], F32, tag="tps")
            nc.tensor.transpose(hT_ps[:], hs_[:], ident[:B, :B])
            hT_sh = cst.tile([128, B], F32)
            nc.scalar.copy(hT_sh[:], hT_ps[:])
            nc.sync.dma_start(out_h[:], hT_sh[:])

            cc_h_in = dram.tile([128, B], F32)
            cc_h_out = dram.tile([H, B], F32, addr_space="Shared")
            nc.sync.dma_start(cc_h_in[:], hT_sh[:])
            nc.gpsimd.collective_compute(
                "AllGather", mybir.AluOpType.bypass, replica_groups=groups,
                ins=[cc_h_in.opt()], outs=[cc_h_out.opt()],
            )
            hT_sb = cst.tile([128, KT, B], F32)
            h_src = cc_h_out.rearrange("(k p) b -> p k b", p=128)
            for i in range(4):
                k0, k1 = 2 * i, 2 * i + 2
                nc.sync.dma_start(hT_sb[:, k0:k1, :], h_src[:, k0:k1, :])
            hTb_sb = cst.tile([128, KT, B], BF16)
            nc.vector.tensor_copy(hTb_sb[:], hT_sb[:])

            # ---------- unembed (bf16): logits chunks + exp sums ----------
            lgs_sb = cst.tile([B, VS], F32)
            sums = cst.tile([B, NVC], F32)
            for j in range(NVC):
                nj = min(VCH, VS - j * VCH)
                wo_t = wst.tile([128, KT, VCH], BF16, tag="wo", bufs=4)
                for i in range(4):
                    k0, k1 = 2 * i, 2 * i + 2
                    dh = nc.sync.dma_start(wo_t[:, k0:k1, :nj],
                                           wo_ap[:, k0:k1, j * VCH:j * VCH + nj])
                    if j < 4:
                        # hold the prefetch until the critical-path DMAs landed
                        bass._add_dep_helper(dh.ins, qk_copy_inst.ins,
                                             reason="delay wo prefetch")
                wob_t = wst.tile([1, VCH], BF16, tag="wob", bufs=2)
                nc.sync.dma_start(wob_t[:, :nj], wob[:, j * VCH:j * VCH + nj])
                lg_ps = psl.tile([B, VCH], F32, tag="lgps")
                for h0 in range(0, nj, 512):
                    h1 = min(nj, h0 + 512)
                    for k in range(KT):
                        nc.tensor.matmul(lg_ps[:, h0:h1], lhsT=hTb_sb[:, k, :],
                                         rhs=wo_t[:, k, h0:h1], start=(k == 0), stop=False)
                    nc.tensor.matmul(lg_ps[:, h0:h1], lhsT=ones_bf[:], rhs=wob_t[:, h0:h1],
                                     start=False, stop=True)
                nc.scalar.copy(lgs_sb[:, j * VCH:j * VCH + nj], lg_ps[:, :nj])
                scr = wst.tile([B, VCH], F32, tag="scr", bufs=2)
                nc.scalar.activation(scr[:, :nj], lg_ps[:, :nj], AF.Exp,
                                     accum_out=sums[:, j:j + 1])

            # ---------- global logsumexp ----------
            ssum = cst.tile([B, 1], F32)
            nc.vector.reduce_sum(ssum[:], sums[:, 0:NVC], axis=mybir.AxisListType.X)
            cc_st_in = dram.tile([B, 1], F32)
            cc_st_out = dram.tile([M * B, 1], F32, addr_space="Shared")
            nc.sync.dma_start(cc_st_in[:], ssum[:])
            nc.gpsimd.collective_compute(
                "AllGather", mybir.AluOpType.bypass, replica_groups=groups,
                ins=[cc_st_in.opt()], outs=[cc_st_out.opt()],
            )
            st_sb = cst.tile([B, M], F32)
            nc.sync.dma_start(st_sb[:], cc_st_out.rearrange("(m b) o -> b (m o)", b=B))
            gsum = cst.tile([B, 1], F32)
            nc.vector.reduce_sum(gsum[:], st_sb[:], axis=mybir.AxisListType.X)
            lse = cst.tile([B, 1], F32)
            nc.scalar.activation(lse[:], gsum[:], AF.Ln)
            nlse = cst.tile([B, 1], F32)
            nc.vector.tensor_scalar_mul(nlse[:], lse[:], -1.0)

            # ---------- final: log_probs = logits - lse (DVE/ACT split) ------
            nsl = 8
            step = -(-VS // nsl)
            for s in range(nsl):
                lo = s * step
                hi = min(VS, lo + step)
                if s % 2 == 0:
                    nc.vector.tensor_scalar_add(lgs_sb[:, lo:hi], lgs_sb[:, lo:hi],
                                                nlse[:, 0:1])
                else:
                    nc.scalar.activation(lgs_sb[:, lo:hi], lgs_sb[:, lo:hi],
                                         AF.Identity, bias=nlse[:, 0:1])
                nc.sync.dma_start(out_logp[:, lo:hi], lgs_sb[:, lo:hi])

    nc.compile()
    return nc


def make_in_maps(input, h_prev, encoder_outputs, emb, w_x, b_x, w_h, b_h,
                 key_w, key_b, value_w, value_b, query_w, query_b, out_w, out_b):
    f = lambda x: np.ascontiguousarray(np.asarray(x, dtype=np.float32))
    bf = lambda x: np.ascontiguousarray(np.asarray(x, dtype=np.float32).astype(BF))
    idx = np.asarray(input).reshape(B).astype(np.int64)
    h_prev = f(h_prev).reshape(B, H)
    enc = f(encoder_outputs)
    w_x, b_x, w_h, b_h = f(w_x), f(b_x), f(w_h), f(b_h)
    key_w, key_b = f(key_w), f(key_b)
    value_w, value_b = f(value_w), f(value_b)
    query_w, query_b = f(query_w), f(query_b)
    out_w, out_b = f(out_w), f(out_b)

    h_prevT = bf(h_prev.T)                                         # [H, B]
    wq_s = bf(query_w / H)
    bqT = np.ascontiguousarray((query_b / H).reshape(KT, 128).T)   # [128, KT]
    bk8 = bf((key_b / M).reshape(KT, 128).T)
    bxh = b_x + b_h
    emb_bf = np.asarray(emb, dtype=np.float32).astype(BF)
    fold_in = np.vstack([np.eye(B, dtype=np.float32)] * 2).astype(BF)  # [128, B]

    in_maps = []
    for m in range(M):
        # embedding row shard
        r0 = m * VR
        nrow = max(0, min(VR, V - r0))
        emb_sh = np.zeros((VR, H), BF)
        emb_sh[:nrow] = emb_bf[r0:r0 + nrow]
        own = (idx >= r0) & (idx < r0 + nrow)
        idx_loc = np.clip(idx - r0, 0, VR - 1).astype(np.int32).reshape(B, 1)
        gmask = own.astype(np.float32).reshape(B, 1)

        # attention shard
        s = m * DS
        enc_sh = bf(np.concatenate([enc[:, :, s:s + 128], enc[:, :, s + 128:s + DS]], axis=0))
        wkt = bf(key_w[s:s + DS, :].T)                             # [H, DS]
        wv = bf(value_w[s:s + DS, :])                              # [DS, H]
        bv_sh = np.ascontiguousarray(value_b[m * 128:(m + 1) * 128].reshape(128, 1))

        # gate column shard
        cols = np.concatenate([np.arange(g * H + m * 128, g * H + (m + 1) * 128)
                               for g in range(4)])
        wxa = np.zeros((H + 128, GS), BF)
        wxa[:H] = w_x[:, cols].astype(BF)
        wxa[H] = bxh[cols].astype(BF)
        wh_sh = bf(w_h[:, cols])

        # vocab column shard
        c0 = m * VS
        ncol = max(0, min(VS, V - c0))
        wo = np.zeros((H, VS), BF)
        wo[:, :ncol] = out_w[:, c0:c0 + ncol].astype(BF)
        wob = np.full((1, VS), NEG_PAD, BF)
        wob[0, :ncol] = out_b[c0:c0 + ncol].astype(BF)

        in_maps.append({
            "idx_loc": idx_loc, "gmask": gmask, "emb_sh": emb_sh,
            "h_prevT": h_prevT, "enc_sh": enc_sh,
            "wq": wq_s, "bqT": bqT, "wkt": wkt, "bk8": bk8,
            "wv": wv, "bv_sh": bv_sh, "wxa": wxa, "wh": wh_sh,
            "wo": wo, "wob": wob, "fold_in": fold_in,
        })
    return in_maps


_NC_CACHE = None


def _get_nc():
    global _NC_CACHE
    if _NC_CACHE is None:
        _NC_CACHE = build_nc()
    return _NC_CACHE


def kernel(**inputs):
    global LAST_EXEC_NS
    nc = _get_nc()
    in_maps = make_in_maps(**inputs)
    trace = bool(int(os.environ.get("KERNEL_TRACE", "0")))
    res = run_bass_kernel_spmd(nc, in_maps, core_ids=list(range(M)), trace=trace)
    LAST_EXEC_NS = res.exec_time_ns
    outs = res.results

    logp = np.concatenate([outs[m]["out_logp"] for m in range(M)], axis=1)[:, :V]
    hT = np.concatenate([outs[m]["out_h"] for m in range(M)], axis=0)   # [H, B]
    attn = outs[0]["out_attn"]
    return (
        logp.reshape(B, 1, V).astype(np.float32),
        np.ascontiguousarray(hT.T).reshape(B, 1, H).astype(np.float32),
        attn.reshape(B, 1, L).astype(np.float32),
    )


if __name__ == "__main__":
    from ref_np import setup_inputs_np
    ins = setup_inputs_np(0)
    outs = kernel(**ins)
    print([o.shape for o in outs], "exec_ns:", LAST_EXEC_NS)


# revision 29
# speedup vs baseline: 1.1156x; 1.1030x over previous
"""AttnDecoderRNN step on 8 Trainium2 NeuronCores (Bass/Tile SPMD kernel).

Sharding strategy (per the tensor-parallel hint):
  - Attention inner dim D=2H=2048 sharded 256/core: WkT & Wv row-shards; scores
    via batched dot products on DVE (batch on partitions, packed 2x64); partial
    scores folded with a constant [I;I] matmul, AllGathered (fused with the
    embedding gather payload) and locally combined; context partials exchanged
    with an AllToAll and locally reduced into the core's h-shard.
  - Embedding table row-sharded (vocab) across cores; local indirect-DMA gather
    + mask rides the scores AllGather (bitcast payload), reassembled locally.
  - LSTM gate columns sharded 128/core (all 4 gates); h_next AllGathered.
  - Output projection column-sharded 6283 vocab cols/core; log_softmax via local
    exp-sums AllGathered and combined on every core.

Precision: weights are bf16 (storage + TensorE), accumulation f32; softmax,
LSTM pointwise, logits, and all outputs are f32.

kernel(**inputs) takes FULL numpy inputs, returns (log_probs, h_next, attn).
"""
import os
import sys

for _p in ("/opt/trn_rl_repo", "/root/.axon_site/_ro/trn_rl_repo"):
    if os.path.isdir(_p) and _p not in sys.path:
        sys.path.insert(0, _p)

import numpy as np
import ml_dtypes

import concourse.bass as bass
import concourse.mybir as mybir
import concourse.tile as tile
from concourse import bacc
from concourse.bass_utils import run_bass_kernel_spmd
from concourse.masks import make_identity

F32 = mybir.dt.float32
BF16 = mybir.dt.bfloat16
I32 = mybir.dt.int32
U8 = mybir.dt.uint8
AF = mybir.ActivationFunctionType
BF = ml_dtypes.bfloat16

B, L, H, V = 64, 50, 1024, 50257
M = 8                      # cores
DS = 2 * H // M            # 256  attention-dim shard
VS = -(-V // M)            # 6283 vocab cols per core (8*6283 = 50264)
VR = VS                    # emb table rows per core
GS = 4 * 128               # 512 gate cols per core (128 per gate)
KT = H // 128              # 8 k-tiles over H
NEG_PAD = -30.0            # logit value for padded vocab columns
VCH = 1024                 # unembed chunk width
SM_W = 188                 # packed smalls width (bytes)

LAST_EXEC_NS = None        # test harness reads this after a traced call


def build_nc():
    nc = bacc.Bacc("TRN2", target_bir_lowering=False, debug=False, num_devices=M)

    def din(name, shape, dtype=F32):
        return nc.dram_tensor(name, shape, dtype, kind="ExternalInput")

    # ---- per-core inputs (host pre-sharded) ----
    smalls = din("smalls", [128, SM_W], U8)     # packed bqT|bv|fold|bk8|gmask|idx
    emb_sh = din("emb_sh", [VR, H], BF16)       # emb row-shard (zero padded)
    h_prevT = din("h_prevT", [H, B], BF16)      # h_prev transposed (bf16)
    enc_sh = din("enc_sh", [128, L, 128], BF16) # [(2 halves x 64b), L, 128d]
    wq = din("wq", [H, H], BF16)                # query_w / H (scale folded)
    wkt = din("wkt", [H, DS], BF16)             # key_w[shard,:]^T
    wv = din("wv", [DS, H], BF16)               # value_w[shard,:]
    wxa = din("wxa", [H + 128, GS], BF16)       # w_x cols + bias row (augmented)
    wh = din("wh", [H, GS], BF16)               # w_h cols
    wo = din("wo", [H, VS], BF16)               # out_w vocab-col shard
    wob = din("wob", [1, VS], BF16)             # out_b shard (pad = NEG_PAD)

    # ---- outputs ----
    out_logp = nc.dram_tensor("out_logp", [B, VS], F32, kind="ExternalOutput")
    out_h = nc.dram_tensor("out_h", [128, B], F32, kind="ExternalOutput")
    out_attn = nc.dram_tensor("out_attn", [B, L], F32, kind="ExternalOutput")

    groups = [list(range(M))]
    NVC = -(-VS // VCH)  # 7 vocab chunks

    wq_ap = wq.ap().rearrange("(k p) h -> p k h", p=128)
    wkt_ap = wkt.ap().rearrange("(k p) d -> p k d", p=128)
    wv_ap = wv.ap().rearrange("(k p) h -> p k h", p=128)
    wxa_ap = wxa.ap().rearrange("(k p) j -> p k j", p=128)
    wh_ap = wh.ap().rearrange("(k p) j -> p k j", p=128)
    wo_ap = wo.ap().rearrange("(k p) v -> p k v", p=128)

    with tile.TileContext(nc) as tc:
        with (
            tc.tile_pool(name="cst", bufs=1) as cst,
            tc.tile_pool(name="wst", bufs=2) as wst,
            tc.tile_pool(name="wbufp", bufs=1) as wbufp,
            tc.tile_pool(name="ps", bufs=2, space="PSUM") as ps,
            tc.tile_pool(name="psl", bufs=2, space="PSUM") as psl,
            tc.tile_pool(name="dram", bufs=1, space="DRAM") as dram,
        ):
            # ---------- barrier collective: absorbs core-dispatch skew and
            # ncfw first-call overhead before the real collectives ----------
            cc_bar_in = dram.tile([1, 4], I32)
            cc_bar_out = dram.tile([M, 4], I32, addr_space="Shared")
            nc.gpsimd.collective_compute(
                "AllGather", mybir.AluOpType.bypass, replica_groups=groups,
                ins=[cc_bar_in.opt()], outs=[cc_bar_out.opt()],
            )

            # ---------- resident loads ----------
            sm_sb = cst.tile([128, SM_W], U8)
            nc.sync.dma_start(sm_sb[:], smalls[:])
            bqT_v = sm_sb[:, 0:32].bitcast(F32)         # [128, 8]
            bv_v = sm_sb[:, 32:36].bitcast(F32)         # [128, 1]
            fold_v = sm_sb[:, 36:164].bitcast(BF16)     # [128, 64]
            bk8_v = sm_sb[:, 164:180].bitcast(BF16)     # [128, 8]
            gmask_v = sm_sb[0:B, 180:184].bitcast(F32)  # [64, 1]
            idx_v = sm_sb[0:B, 184:188].bitcast(I32)    # [64, 1]

            hpTb_sb = cst.tile([128, KT, B], BF16)
            nc.sync.dma_start(hpTb_sb[:], h_prevT.ap().rearrange("(k p) b -> p k b", p=128))
            enc_sb = cst.tile([128, L, 128], BF16)
            for i in range(8):
                l0, l1 = (L * i) // 8, (L * (i + 1)) // 8
                nc.sync.dma_start(enc_sb[:, l0:l1, :], enc_sh[:, l0:l1, :])
            wkt_sb = cst.tile([128, KT, DS], BF16)
            nc.sync.dma_start(wkt_sb[:, 0:4, :], wkt_ap[:, 0:4, :])
            nc.sync.dma_start(wkt_sb[:, 4:8, :], wkt_ap[:, 4:8, :])
            wv_sb = cst.tile([128, 2, H], BF16)
            nc.sync.dma_start(wv_sb[:, 0, :], wv_ap[:, 0, :])
            nc.sync.dma_start(wv_sb[:, 1, :], wv_ap[:, 1, :])
            wxa_sb = cst.tile([128, KT + 1, GS], BF16)
            nc.sync.dma_start(wxa_sb[:, 0:5, :], wxa_ap[:, 0:5, :])
            nc.sync.dma_start(wxa_sb[:, 5:9, :], wxa_ap[:, 5:9, :])
            wh_sb = cst.tile([128, KT, GS], BF16)
            nc.sync.dma_start(wh_sb[:, 0:4, :], wh_ap[:, 0:4, :])
            nc.sync.dma_start(wh_sb[:, 4:8, :], wh_ap[:, 4:8, :])
            wob_sb = cst.tile([1, VS], BF16)
            nc.sync.dma_start(wob_sb[:], wob[:])

            ident = cst.tile([128, 128], F32)
            make_identity(nc, ident[:])
            ident_bf = cst.tile([128, 128], BF16)
            make_identity(nc, ident_bf[:])
            ones_bf = cst.tile([1, B], BF16)
            nc.vector.memset(ones_bf[:], 1.0)

            # ---------- q = h_prev @ (Wq/H)  (wq loaded via scalar engine) ----
            wq_sb = cst.tile([128, KT, H], BF16)
            for k in range(KT):
                nc.scalar.dma_start(wq_sb[:, k, :], wq_ap[:, k, :])
            q_ps0 = ps.tile([B, 512], F32, tag="qps")
            q_ps1 = ps.tile([B, 512], F32, tag="qps")
            for k in range(KT):
                for half, qp in ((0, q_ps0), (1, q_ps1)):
                    nc.tensor.matmul(
                        qp[:], lhsT=hpTb_sb[:, k, :],
                        rhs=wq_sb[:, k, half * 512:(half + 1) * 512],
                        start=(k == 0), stop=(k == KT - 1),
                    )
            q_sb = cst.tile([B, H], F32)
            nc.scalar.copy(q_sb[:, 0:512], q_ps0[:])
            nc.scalar.copy(q_sb[:, 512:1024], q_ps1[:])

            # qT (bf16) with per-partition bias bq/H
            qT_sb = cst.tile([128, KT, B], BF16)
            for k in range(KT):
                tp = ps.tile([128, B], F32, tag="tps")
                nc.tensor.transpose(tp[:], q_sb[:, k * 128:(k + 1) * 128], ident[:B, :B])
                nc.scalar.activation(qT_sb[:, k, :], tp[:], AF.Identity, bias=bqT_v[:, k:k + 1])

            # ---------- qk packed [ (2 halves x 64b), 128 d ] ----------
            qk_ps = ps.tile([128, 128], F32, tag="qps")
            for k in range(KT):
                nc.tensor.matmul(qk_ps[:B, :], lhsT=qT_sb[:, k, :], rhs=wkt_sb[:, k, 0:128],
                                 start=(k == 0), stop=(k == KT - 1))
            for k in range(KT):
                nc.tensor.matmul(qk_ps[B:, :], lhsT=qT_sb[:, k, :], rhs=wkt_sb[:, k, 128:256],
                                 start=(k == 0), stop=(k == KT - 1), tile_position=(0, 64))
            qk_sb = cst.tile([128, 128], BF16)
            qk_copy_inst = nc.scalar.copy(qk_sb[:], qk_ps[:])

            # qb = q' . (key_b/M)   [B, 1]
            qb_ps = ps.tile([B, 1], F32, tag="tps")
            for k in range(KT):
                nc.tensor.matmul(qb_ps[:], lhsT=qT_sb[:, k, :], rhs=bk8_v[:, k:k + 1],
                                 start=(k == 0), stop=(k == KT - 1))
            qb_sb = cst.tile([B, 1], F32)
            nc.scalar.copy(qb_sb[:], qb_ps[:])

            # ---------- scores partial = sum_d enc*qk (tree over d) ----------
            wbuf = wbufp.tile([128, L, 128], BF16, tag="wbuf")
            nc.vector.tensor_mul(wbuf[:], enc_sb[:], qk_sb[:, None, :].to_broadcast([128, L, 128]))
            n = 128
            while n > 1:
                hn = n // 2
                nc.vector.tensor_add(wbuf[:, :, 0:hn], wbuf[:, :, 0:hn], wbuf[:, :, hn:n])
                n = hn
            sred_c = cst.tile([128, L], BF16)
            nc.vector.tensor_copy(sred_c[:], wbuf[:, :, 0])
            # fold the packed d-halves across partitions with the [I;I] matmul;
            # qb rides in as the ACT copy bias
            fold_ps = ps.tile([B, L], F32, tag="tps")
            nc.tensor.matmul(fold_ps[:], lhsT=fold_v[:], rhs=sred_c[:], start=True, stop=True)
            sc_part = cst.tile([B, L], F32)
            nc.scalar.activation(sc_part[:], fold_ps[:], AF.Identity, bias=qb_sb[:, 0:1])

            # ---------- embedding gather (fused into the scores AllGather:
            # cols [0:L) carry score partials f32, cols [L:L+H/2) carry the
            # masked bf16 embedding gather, bitcast to f32 pairs) ------------
            gat = cst.tile([B, H], BF16)
            nc.gpsimd.indirect_dma_start(
                out=gat[:], out_offset=None,
                in_=emb_sh[:],
                in_offset=bass.IndirectOffsetOnAxis(ap=idx_v[:, :1], axis=0),
            )
            gat_m = cst.tile([B, H], BF16)
            nc.vector.tensor_scalar_mul(gat_m[:], gat[:], gmask_v[:, 0:1])

            EW = H // 2  # 512 f32 words carrying 1024 bf16 embedding values
            cc_sc_in = dram.tile([B, L + EW], F32)
            cc_sc_out = dram.tile([M * B, L + EW], F32, addr_space="Shared")
            nc.sync.dma_start(cc_sc_in[:, L:], gat_m[:].bitcast(F32))
            nc.sync.dma_start(cc_sc_in[:, 0:L], sc_part[:])
            nc.gpsimd.collective_compute(
                "AllGather", mybir.AluOpType.bypass, replica_groups=groups,
                ins=[cc_sc_in.opt()], outs=[cc_sc_out.opt()],
            )

            # ---------- combine gathered score partials + softmax ----------
            sc8 = cst.tile([128, M, L], F32)
            sc_src = cc_sc_out.rearrange("(m b) e -> b m e", b=B)
            nc.sync.dma_start(sc8[:B, :, :], sc_src[:, :, 0:L])
            nc.sync.dma_start(sc8[B:, :, :], sc_src[:, :, 0:L])
            n = M
            while n > 1:
                hn = n // 2
                nc.vector.tensor_add(sc8[:, 0:hn, :], sc8[:, 0:hn, :], sc8[:, hn:n, :])
                n = hn
            attn_raw = sc8[:, 0, :]
            nmax = cst.tile([128, 1], F32)
            nc.vector.reduce_max(nmax[:], attn_raw, axis=mybir.AxisListType.X, negate=True)
            attn_e = cst.tile([128, L], F32)
            sexp = cst.tile([128, 1], F32)
            nc.scalar.activation(attn_e[:], attn_raw, AF.Exp,
                                 bias=nmax[:, 0:1], accum_out=sexp[:, 0:1])
            rcp = cst.tile([128, 1], F32)
            nc.vector.reciprocal(rcp[:], sexp[:])
            attn_bf = cst.tile([128, L], BF16)
            nc.vector.tensor_scalar_mul(attn_bf[:], attn_e[:], rcp[:, 0:1])
            attn_f = cst.tile([B, L], F32)
            nc.vector.tensor_scalar_mul(attn_f[:], attn_e[:B, :], rcp[:B, 0:1])
            nc.sync.dma_start(out_attn[:], attn_f[:])

            # ---------- t = attn @ enc  (packed, tree reduce over L) ----------
            wbuf2 = wbufp.tile([128, L, 128], BF16, tag="wbuf")
            nc.vector.tensor_mul(wbuf2[:], enc_sb[:],
                                 attn_bf[:, :, None].to_broadcast([128, L, 128]))
            n = L
            while n > 1:
                hn = n // 2
                r = n - hn
                nc.vector.tensor_add(wbuf2[:, 0:hn, :], wbuf2[:, 0:hn, :], wbuf2[:, r:n, :])
                n = r

            # tT [d(2x128), b]  (bf16)
            tT_sb = cst.tile([128, 2, B], BF16)
            for g in range(2):
                tpb = ps.tile([128, B], BF16, tag="tps", name=f"tpb_{g}")
                nc.tensor.transpose(tpb[:], wbuf2[g * B:(g + 1) * B, 0, :],
                                    ident_bf[g * B:(g + 1) * B, g * B:(g + 1) * B])
                nc.scalar.copy(tT_sb[:, g, :], tpb[:])

            # embedding blocks from the fused AllGather: masked per-core
            # contributions; summing just reassembles rows (exact in bf16)
            em8 = cst.tile([B, M, EW], F32)
            for i in range(4):
                m0, m1 = 2 * i, 2 * i + 2
                nc.sync.dma_start(em8[:, m0:m1, :], sc_src[:, m0:m1, L:])
            em8b = em8[:].bitcast(BF16)           # [B, M, H]
            n = M
            while n > 1:
                hn = n // 2
                nc.vector.tensor_add(em8b[:, 0:hn, :], em8b[:, 0:hn, :], em8b[:, hn:n, :])
                n = hn
            embedded = em8b[:, 0, :]              # [B, H] bf16 view

            # ---------- cT partial -> AllToAll -> local reduce ----------
            ctb_sb = cst.tile([128, KT, B], F32)
            for mo in range(KT):
                cp = ps.tile([128, B], F32, tag="tps", name=f"cp_{mo}")
                for kd in range(2):
                    nc.tensor.matmul(cp[:], lhsT=wv_sb[:, kd, mo * 128:(mo + 1) * 128],
                                     rhs=tT_sb[:, kd, :], start=(kd == 0), stop=(kd == 1))
                nc.scalar.copy(ctb_sb[:, mo, :], cp[:])

            cc_ct_in = dram.tile([H, B], F32)
            cc_ct_out = dram.tile([H, B], F32)
            ct_dst = cc_ct_in.rearrange("(k p) b -> p k b", p=128)
            nc.sync.dma_start(ct_dst[:, 0:4, :], ctb_sb[:, 0:4, :])
            nc.sync.dma_start(ct_dst[:, 4:8, :], ctb_sb[:, 4:8, :])
            nc.gpsimd.collective_compute(
                "AllToAll", mybir.AluOpType.bypass, replica_groups=groups,
                ins=[cc_ct_in.opt()], outs=[cc_ct_out.opt()],
            )
            ct8_sb = cst.tile([128, M, B], F32)
            ct_src = cc_ct_out.rearrange("(m p) b -> p m b", p=128)
            nc.sync.dma_start(ct8_sb[:, 0:4, :], ct_src[:, 0:4, :])
            nc.sync.dma_start(ct8_sb[:, 4:8, :], ct_src[:, 4:8, :])
            n = M
            while n > 1:
                hn = n // 2
                nc.vector.tensor_add(ct8_sb[:, 0:hn, :], ct8_sb[:, 0:hn, :], ct8_sb[:, hn:n, :])
                n = hn
            ct_b = cst.tile([128, B], F32)
            nc.scalar.activation(ct_b[:], ct8_sb[:, 0, :], AF.Identity, bias=bv_v[:, 0:1])

            # c_prev shard in [b, j] layout
            cb_ps = ps.tile([B, 128], F32, tag="tps")
            nc.tensor.transpose(cb_ps[:], ct_b[:], ident[:, :])
            cb_sb = cst.tile([B, 128], F32)
            nc.scalar.copy(cb_sb[:], cb_ps[:])

            # ---------- embT build (bf16) ----------
            embT_sb = cst.tile([128, KT + 1, B], BF16)
            for k in range(KT):
                tpe = ps.tile([128, B], BF16, tag="tps", name=f"tpe_{k}")
                nc.tensor.transpose(tpe[:], embedded[:, k * 128:(k + 1) * 128], ident_bf[:B, :B])
                nc.scalar.copy(embT_sb[:, k, :], tpe[:])
            nc.vector.memset(embT_sb[:, KT, :], 0.0)
            nc.vector.memset(embT_sb[0:1, KT, :], 1.0)

            # ---------- gates [b, 4x128] ----------
            g_ps = ps.tile([B, GS], F32, tag="qps")
            for k in range(KT + 1):
                nc.tensor.matmul(g_ps[:], lhsT=embT_sb[:, k, :], rhs=wxa_sb[:, k, :],
                                 start=(k == 0), stop=False)
            for k in range(KT):
                nc.tensor.matmul(g_ps[:], lhsT=hpTb_sb[:, k, :], rhs=wh_sb[:, k, :],
                                 start=False, stop=(k == KT - 1))
            g_sb = cst.tile([B, GS], F32)
            nc.scalar.copy(g_sb[:], g_ps[:])

            f_s = cst.tile([B, 128], F32)
            i_s = cst.tile([B, 128], F32)
            cbar = cst.tile([B, 128], F32)
            o_s = cst.tile([B, 128], F32)
            nc.scalar.activation(f_s[:], g_sb[:, 0:128], AF.Sigmoid)
            nc.scalar.activation(i_s[:], g_sb[:, 128:256], AF.Sigmoid)
            nc.scalar.activation(cbar[:], g_sb[:, 256:384], AF.Tanh)
            nc.scalar.activation(o_s[:], g_sb[:, 384:512], AF.Sigmoid)

            t1 = cst.tile([B, 128], F32)
            nc.vector.tensor_mul(t1[:], f_s[:], cb_sb[:])
            t2 = cst.tile([B, 128], F32)
            nc.vector.tensor_mul(t2[:], i_s[:], cbar[:])
            cn = cst.tile([B, 128], F32)
            nc.vector.tensor_add(cn[:], t1[:], t2[:])
            tc_ = cst.tile([B, 128], F32)
            nc.scalar.activation(tc_[:], cn[:], AF.Tanh)
            hs_ = cst.tile([B, 128], F32)
            nc.vector.tensor_mul(hs_[:], o_s[:], tc_[:])

            hT_ps = ps.tile([128, B], F32, tag="tps")
            nc.tensor.transpose(hT_ps[:], hs_[:], ident[:B, :B])
            hT_sh = cst.tile([128, B], F32)
            nc.scalar.copy(hT_sh[:], hT_ps[:])
            nc.sync.dma_start(out_h[:], hT_sh[:])

            cc_h_in = dram.tile([128, B], F32)
            cc_h_out = dram.tile([H, B], F32, addr_space="Shared")
            nc.sync.dma_start(cc_h_in[:], hT_sh[:])
            nc.gpsimd.collective_compute(
                "AllGather", mybir.AluOpType.bypass, replica_groups=groups,
                ins=[cc_h_in.opt()], outs=[cc_h_out.opt()],
            )
            hT_sb = cst.tile([128, KT, B], F32)
            h_src = cc_h_out.rearrange("(k p) b -> p k b", p=128)
            nc.sync.dma_start(hT_sb[:, 0:4, :], h_src[:, 0:4, :])
            nc.sync.dma_start(hT_sb[:, 4:8, :], h_src[:, 4:8, :])
            hTb_sb = cst.tile([128, KT, B], BF16)
            nc.vector.tensor_copy(hTb_sb[:], hT_sb[:])

            # ---------- unembed (bf16): logits chunks + exp sums ----------
            # chunks 0-3 prefetch early via the scalar engine (gated past the
            # critical q/qk DMAs); chunks 4-6 stream from sync in-loop.
            lgs_sb = cst.tile([B, VS], F32)
            sums = cst.tile([B, NVC], F32)
            wo_ts = [wst.tile([128, KT, VCH], BF16, tag="wo", bufs=3, name=f"wo_{j}")
                     for j in range(NVC)]
            for j in range(3):
                nj = min(VCH, VS - j * VCH)
                dh = nc.scalar.dma_start(wo_ts[j][:, :, :nj],
                                         wo_ap[:, :, j * VCH:j * VCH + nj])
                bass._add_dep_helper(dh.ins, qk_copy_inst.ins,
                                     reason="delay wo prefetch past critical DMAs")
            for j in range(NVC):
                nj = min(VCH, VS - j * VCH)
                wo_t = wo_ts[j]
                if j >= 3:
                    for i in range(4):
                        k0, k1 = 2 * i, 2 * i + 2
                        nc.sync.dma_start(wo_t[:, k0:k1, :nj],
                                          wo_ap[:, k0:k1, j * VCH:j * VCH + nj])
                lg_ps = psl.tile([B, VCH], F32, tag="lgps")
                for h0 in range(0, nj, 512):
                    h1 = min(nj, h0 + 512)
                    for k in range(KT):
                        nc.tensor.matmul(lg_ps[:, h0:h1], lhsT=hTb_sb[:, k, :],
                                         rhs=wo_t[:, k, h0:h1], start=(k == 0), stop=False)
                    nc.tensor.matmul(lg_ps[:, h0:h1], lhsT=ones_bf[:],
                                     rhs=wob_sb[:, j * VCH + h0:j * VCH + h1],
                                     start=False, stop=True)
                nc.scalar.copy(lgs_sb[:, j * VCH:j * VCH + nj], lg_ps[:, :nj])
                scr = wst.tile([B, VCH], F32, tag="scr", bufs=1)
                nc.scalar.activation(scr[:, :nj], lg_ps[:, :nj], AF.Exp,
                                     accum_out=sums[:, j:j + 1])

            # ---------- global logsumexp ----------
            ssum = cst.tile([B, 1], F32)
            nc.vector.reduce_sum(ssum[:], sums[:, 0:NVC], axis=mybir.AxisListType.X)
            cc_st_in = dram.tile([B, 1], F32)
            cc_st_out = dram.tile([M * B, 1], F32, addr_space="Shared")
            nc.sync.dma_start(cc_st_in[:], ssum[:])
            nc.gpsimd.collective_compute(
                "AllGather", mybir.AluOpType.bypass, replica_groups=groups,
                ins=[cc_st_in.opt()], outs=[cc_st_out.opt()],
            )
            st_sb = cst.tile([B, M], F32)
            nc.sync.dma_start(st_sb[:], cc_st_out.rearrange("(m b) o -> b (m o)", b=B))
            gsum = cst.tile([B, 1], F32)
            nc.vector.reduce_sum(gsum[:], st_sb[:], axis=mybir.AxisListType.X)
            lse = cst.tile([B, 1], F32)
            nc.scalar.activation(lse[:], gsum[:], AF.Ln)
            nlse = cst.tile([B, 1], F32)
            nc.vector.tensor_scalar_mul(nlse[:], lse[:], -1.0)

            # ---------- final: log_probs = logits - lse (DVE/ACT split) ------
            nsl = 8
            step = -(-VS // nsl)
            for s in range(nsl):
                lo = s * step
                hi = min(VS, lo + step)
                if s % 2 == 0:
                    nc.vector.tensor_scalar_add(lgs_sb[:, lo:hi], lgs_sb[:, lo:hi],
                                                nlse[:, 0:1])
                else:
                    nc.scalar.activation(lgs_sb[:, lo:hi], lgs_sb[:, lo:hi],
                                         AF.Identity, bias=nlse[:, 0:1])
                nc.sync.dma_start(out_logp[:, lo:hi], lgs_sb[:, lo:hi])

    nc.compile()
    return nc


def make_in_maps(input, h_prev, encoder_outputs, emb, w_x, b_x, w_h, b_h,
                 key_w, key_b, value_w, value_b, query_w, query_b, out_w, out_b):
    f = lambda x: np.ascontiguousarray(np.asarray(x, dtype=np.float32))
    bf = lambda x: np.ascontiguousarray(np.asarray(x, dtype=np.float32).astype(BF))
    idx = np.asarray(input).reshape(B).astype(np.int64)
    h_prev = f(h_prev).reshape(B, H)
    enc = f(encoder_outputs)
    w_x, b_x, w_h, b_h = f(w_x), f(b_x), f(w_h), f(b_h)
    key_w, key_b = f(key_w), f(key_b)
    value_w, value_b = f(value_w), f(value_b)
    query_w, query_b = f(query_w), f(query_b)
    out_w, out_b = f(out_w), f(out_b)

    h_prevT = bf(h_prev.T)                                         # [H, B]
    wq_s = bf(query_w / H)
    bqT = np.ascontiguousarray((query_b / H).reshape(KT, 128).T)   # [128, KT] f32
    bk8 = np.ascontiguousarray((key_b / M).reshape(KT, 128).T).astype(BF)
    bxh = b_x + b_h
    emb_bf = np.asarray(emb, dtype=np.float32).astype(BF)
    fold_m = np.vstack([np.eye(B, dtype=np.float32)] * 2).astype(BF)  # [128, B]

    in_maps = []
    for m in range(M):
        # embedding row shard
        r0 = m * VR
        nrow = max(0, min(VR, V - r0))
        emb_sh = np.zeros((VR, H), BF)
        emb_sh[:nrow] = emb_bf[r0:r0 + nrow]
        own = (idx >= r0) & (idx < r0 + nrow)
        idx_loc = np.clip(idx - r0, 0, VR - 1).astype(np.int32).reshape(B, 1)
        gmask = own.astype(np.float32).reshape(B, 1)

        # packed smalls [128, SM_W] u8: bqT | bv | fold | bk8 | gmask | idx
        bv_sh = np.ascontiguousarray(value_b[m * 128:(m + 1) * 128].reshape(128, 1))
        gmask_full = np.zeros((128, 1), np.float32)
        gmask_full[:B] = gmask
        idx_full = np.zeros((128, 1), np.int32)
        idx_full[:B] = idx_loc
        smalls = np.concatenate([
            bqT.view(np.uint8).reshape(128, 32),
            bv_sh.view(np.uint8).reshape(128, 4),
            fold_m.view(np.uint8).reshape(128, 128),
            bk8.view(np.uint8).reshape(128, 16),
            gmask_full.view(np.uint8).reshape(128, 4),
            idx_full.view(np.uint8).reshape(128, 4),
        ], axis=1)
        assert smalls.shape == (128, SM_W)

        # attention shard
        s = m * DS
        enc_sh = bf(np.concatenate([enc[:, :, s:s + 128], enc[:, :, s + 128:s + DS]], axis=0))
        wkt = bf(key_w[s:s + DS, :].T)                             # [H, DS]
        wv = bf(value_w[s:s + DS, :])                              # [DS, H]

        # gate column shard
        cols = np.concatenate([np.arange(g * H + m * 128, g * H + (m + 1) * 128)
                               for g in range(4)])
        wxa = np.zeros((H + 128, GS), BF)
        wxa[:H] = w_x[:, cols].astype(BF)
        wxa[H] = bxh[cols].astype(BF)
        wh_sh = bf(w_h[:, cols])

        # vocab column shard
        c0 = m * VS
        ncol = max(0, min(VS, V - c0))
        wo = np.zeros((H, VS), BF)
        wo[:, :ncol] = out_w[:, c0:c0 + ncol].astype(BF)
        wob = np.full((1, VS), NEG_PAD, BF)
        wob[0, :ncol] = out_b[c0:c0 + ncol].astype(BF)

        in_maps.append({
            "smalls": np.ascontiguousarray(smalls), "emb_sh": emb_sh,
            "h_prevT": h_prevT, "enc_sh": enc_sh,
            "wq": wq_s, "wkt": wkt, "wv": wv, "wxa": wxa, "wh": wh_sh,
            "wo": wo, "wob": wob,
        })
    return in_maps


_NC_CACHE = None


def _get_nc():
    global _NC_CACHE
    if _NC_CACHE is None:
        _NC_CACHE = build_nc()
    return _NC_CACHE


def kernel(**inputs):
    global LAST_EXEC_NS
    nc = _get_nc()
    in_maps = make_in_maps(**inputs)
    trace = bool(int(os.environ.get("KERNEL_TRACE", "0")))
    res = run_bass_kernel_spmd(nc, in_maps, core_ids=list(range(M)), trace=trace)
    LAST_EXEC_NS = res.exec_time_ns
    outs = res.results

    logp = np.concatenate([outs[m]["out_logp"] for m in range(M)], axis=1)[:, :V]
    hT = np.concatenate([outs[m]["out_h"] for m in range(M)], axis=0)   # [H, B]
    attn = outs[0]["out_attn"]
    return (
        logp.reshape(B, 1, V).astype(np.float32),
        np.ascontiguousarray(hT.T).reshape(B, 1, H).astype(np.float32),
        attn.reshape(B, 1, L).astype(np.float32),
    )


if __name__ == "__main__":
    from ref_np import setup_inputs_np
    ins = setup_inputs_np(0)
    outs = kernel(**ins)
    print([o.shape for o in outs], "exec_ns:", LAST_EXEC_NS)
